# revision 6
# baseline (speedup 1.0000x reference)
"""CTRNN kernel for 8x TRN2 NeuronCores (Bass/Tile), data-parallel over batch.

Math (per timestep, reference):
    xi_t  = x_t @ Wi.T + bi
    h_new = relu(xi_t + h @ Wh.T + bh)
    h     = 0.9*h + 0.1*h_new

Folding 0.1 into the weights/biases (relu is positively homogeneous):
    xi'_t = x_t @ (0.1*Wi).T + 0.1*(bi+bh)      (precomputed GEMM, phase A)
    h     = 0.9*h + relu(xi'_t + h @ (0.1*Wh).T) (recurrence, phase B)

Layouts keep hidden-on-partitions everywhere ("h.T" layout), so the
recurrence needs no transposes: state h is [128p, (chunk c, batch b)] with
hidden = c*128 + p.
"""

import sys

if "/opt/trn_rl_repo" not in sys.path:
    sys.path.insert(0, "/opt/trn_rl_repo")

import numpy as np

import concourse.bass as bass
import concourse.mybir as mybir
from concourse.tile import TileContext
import concourse.tile as _tile_mod
from concourse.bass_utils import run_bass_kernel_spmd

# ---------------------------------------------------------------- constants
ALPHA = 0.1
N_CORES = 8
SEQ, BATCH, INSZ, HID = 512, 128, 256, 1024
BPC = BATCH // N_CORES          # 16 batch elements per core
TB = SEQ * BPC                  # 8192 (t, b) free elements per core
KC = HID // 128                 # 8 hidden chunks
GROUP = 16                      # timesteps per half-body
F32 = mybir.dt.float32

DT_REC = mybir.dt.float32       # recurrence matmul operand dtype
DT_GEMM = mybir.dt.float32      # phase-A GEMM operand dtype

# ------------------------------------------------- walrus sync-wait patches
# This walrus build rejects instructions carrying more than MAX_WAITS sync
# waits. Split excess waits onto same-engine NoOps ahead of the instruction.
MAX_WAITS = 1

_orig_lower_ordered = _tile_mod.TileContext._lower_ordered_insts


def _split_waits_and_lower(self, ordered):
    import bass_rust

    nc = self.nc
    for bb_name, insts in ordered.items():
        new_list = []
        for inst in insts:
            si = getattr(inst, "sync_info", None)
            if si is not None and si.on_wait and len(si.on_wait) > MAX_WAITS:
                waits = list(si.on_wait)
                si.on_wait = waits[:MAX_WAITS]
                rest = waits[MAX_WAITS:]
                for j in range(0, len(rest), MAX_WAITS):
                    nop = bass_rust.InstNoOp(
                        name=nc.get_next_instruction_name(), ins=[], outs=[]
                    )
                    nop.engine = inst.engine
                    nop.sync_info = bass_rust.SyncInfo(
                        on_wait=rest[j : j + MAX_WAITS], on_update=[]
                    )
                    new_list.append(nop)
            new_list.append(inst)
        insts[:] = new_list
    return _orig_lower_ordered(self, ordered)


_tile_mod.TileContext._lower_ordered_insts = _split_waits_and_lower

_orig_drain_and_barrier = _tile_mod.TileContext._drain_and_barrier


def _split_drain_and_barrier(self, tick_clock, wait_clock):
    import bass_rust

    nc = self.nc
    drain_inst = nc.sync.drain()
    _tile_mod.ScopedClock if hasattr(_tile_mod, "ScopedClock") else None
    from concourse.tile import ScopedClock

    wait_clock.add_sem_waits(
        drain_inst.ins, ScopedClock({None: tick_clock.global_clock})
    )
    si = drain_inst.ins.sync_info
    waits = list(si.on_wait) if si is not None else []
    MAXW = 1
    if len(waits) > MAXW:
        si.on_wait = waits[:MAXW]
        for i in range(MAXW, len(waits), MAXW):
            d2 = nc.sync.drain()
            d2.ins.sync_info = bass_rust.SyncInfo(
                on_wait=waits[i : i + MAXW], on_update=[]
            )
    nc.all_engine_barrier()
    popped = nc._tile_sem_poison_stack.pop()
    assert popped is self._sem_poison
    nc.clear_and_free_semaphores(list(self.sems.allocated().values()))
    nc.all_engine_barrier()


_tile_mod.TileContext._drain_and_barrier = _split_drain_and_barrier


# ------------------------------------------------------------ device build
def build_bass():
    nc = bass.Bass()

    xT = nc.dram_tensor("xT", [INSZ, TB], F32, kind="ExternalInput")
    wiT = nc.dram_tensor("wiT", [INSZ, HID], DT_GEMM, kind="ExternalInput")
    whT = nc.dram_tensor("whT", [HID, HID], DT_REC, kind="ExternalInput")
    biasv = nc.dram_tensor("biasv", [HID, 1], F32, kind="ExternalInput")
    h0T = nc.dram_tensor("h0T", [HID, BPC], F32, kind="ExternalInput")
    outT = nc.dram_tensor("outT", [HID, TB], F32, kind="ExternalOutput")

    with TileContext(nc) as tc:
        with (
            tc.tile_pool(name="const", bufs=1) as constp,
            tc.tile_pool(name="dram", bufs=1, space="DRAM") as dramp,
            tc.tile_pool(name="xa", bufs=3) as xa_pool,
            tc.tile_pool(name="psA", bufs=2, space="PSUM") as psA,
            tc.tile_pool(name="evA", bufs=4) as evA,
            tc.tile_pool(name="psB", bufs=4, space="PSUM") as psB,
            tc.tile_pool(name="zr", bufs=6) as zr_pool,
            tc.tile_pool(name="hm", bufs=6) as hm_pool,
            tc.tile_pool(name="st", bufs=1) as st_pool,
        ):
            # ---- persistent SBUF constants
            def dr3(ap2d):
                """DRAM (C*128, F) viewed as [p, c, f]."""
                return ap2d.rearrange("(c p) f -> p c f", p=128)

            def sb3(tile_ap, nch):
                return tile_ap.rearrange("p (c f) -> p c f", c=nch)

            whT_sb = constp.tile([128, KC * HID], DT_REC, tag="whT")
            nc.sync.dma_start(out=sb3(whT_sb[:], KC), in_=dr3(whT[:]))
            wiT_sb = constp.tile([128, 2 * HID], DT_GEMM, tag="wiT")
            nc.sync.dma_start(out=sb3(wiT_sb[:], 2), in_=dr3(wiT[:]))
            bias_sb = constp.tile([128, KC], F32, tag="bias")
            nc.sync.dma_start(out=sb3(bias_sb[:], KC), in_=dr3(biasv[:]))
            h0_sb = constp.tile([128, KC * BPC], F32, tag="h0")
            nc.sync.dma_start(out=sb3(h0_sb[:], KC), in_=dr3(h0T[:]))

            # xi' scratch in DRAM, padded so the steady-state prefetch of the
            # (nonexistent) next group never reads out of bounds.
            XI_PAD = 2 * GROUP * BPC
            xi_tmp = dramp.tile([HID, TB + XI_PAD], F32, tag="xi")

            # ---------------- phase A: xi' = xT.T @ (0.1 Wi).T + bias
            NT = 512                       # (t,b) tile width
            for n in range(TB // NT):
                xt = xa_pool.tile([128, 2 * NT], DT_GEMM, tag="xt")
                nc.sync.dma_start(
                    out=sb3(xt[:], 2), in_=dr3(xT[:, n * NT : (n + 1) * NT])
                )
                for m in range(KC):
                    ps = psA.tile([128, NT], F32, tag="psA")
                    for k in range(2):
                        nc.tensor.matmul(
                            ps[:],
                            lhsT=wiT_sb[:, k * HID + m * 128 : k * HID + (m + 1) * 128],
                            rhs=xt[:, k * NT : (k + 1) * NT],
                            start=(k == 0),
                            stop=(k == 1),
                        )
                    ev = evA.tile([128, NT], F32, tag="ev")
                    nc.scalar.activation(
                        ev[:],
                        ps[:],
                        mybir.ActivationFunctionType.Identity,
                        bias=bias_sb[:, m : m + 1],
                        scale=1.0,
                    )
                    nc.sync.dma_start(
                        out=xi_tmp[m * 128 : (m + 1) * 128, n * NT : (n + 1) * NT],
                        in_=ev[:],
                    )

            # ---------------- phase B: 512-step recurrence
            GF = GROUP * BPC               # 256 free elements per 16-step group
            xi_A = st_pool.tile([128, KC * GF], F32, tag="xiA")
            xi_B = st_pool.tile([128, KC * GF], F32, tag="xiB")
            ob_A = st_pool.tile([128, KC * GF], F32, tag="obA")
            ob_B = st_pool.tile([128, KC * GF], F32, tag="obB")
            # step-boundary moving-operand tiles (stable addresses across the
            # loop back-edge: written by the last step of each body, read by
            # the first step of the next body)
            hm_wrap = (
                st_pool.tile([128, 4 * BPC], DT_REC, name="hmw0", tag="hmw0"),
                st_pool.tile([128, 4 * BPC], DT_REC, name="hmw1", tag="hmw1"),
            )

            def v3(tile_ap, nch=KC):
                return tile_ap[:].rearrange("p (c f) -> p c f", c=nch)

            # prologue: h state = h0, staged where the first body step looks
            nc.vector.tensor_copy(
                out=v3(ob_B)[:, :, (GROUP - 1) * BPC : GROUP * BPC],
                in_=v3(h0_sb, nch=KC),
            )
            for half in range(2):
                nc.vector.tensor_copy(
                    out=hm_wrap[half][:],
                    in_=h0_sb[:, half * 4 * BPC : (half + 1) * 4 * BPC],
                )
            nc.sync.dma_start(out=sb3(xi_A[:], KC), in_=dr3(xi_tmp[:, 0:GF]))

            def do_step(xi_t, ob_cur, prev_slot_src, s, hm_prev, hm_out):
                """One timestep. xi_t/ob_cur: [128, KC*GF] tiles; s: slot in
                ob_cur; prev_slot_src: (tile, slot) for h_{t-1}; hm_prev:
                (lo, hi) moving-operand tiles holding h_{t-1} as DT_REC;
                hm_out: (lo, hi) tiles to fill with h_t."""
                ps_halves = []
                for half in range(2):
                    ps = psB.tile([128, 4 * BPC], F32, tag="psB")
                    ps_halves.append(ps)
                    for mloc in range(4):
                        m = half * 4 + mloc
                        for k in range(KC):
                            nc.tensor.matmul(
                                ps[:, mloc * BPC : (mloc + 1) * BPC],
                                lhsT=whT_sb[
                                    :, k * HID + m * 128 : k * HID + (m + 1) * 128
                                ],
                                rhs=hm_prev[k // 4][
                                    :, (k % 4) * BPC : (k % 4 + 1) * BPC
                                ],
                                start=(k == 0),
                                stop=(k == KC - 1),
                            )
                    # elementwise for this half as soon as its psum is done
                    pv, pslot = prev_slot_src
                    z = zr_pool.tile([128, 4 * BPC], F32, tag="z")
                    nc.vector.tensor_add(
                        out=v3(z, nch=4),
                        in0=v3(ps, nch=4),
                        in1=v3(xi_t)[:, half * 4 : half * 4 + 4, s * BPC : (s + 1) * BPC],
                    )
                    r = zr_pool.tile([128, 4 * BPC], F32, tag="r")
                    nc.scalar.activation(
                        r[:], z[:], mybir.ActivationFunctionType.Relu
                    )
                    nc.vector.scalar_tensor_tensor(
                        out=v3(ob_cur)[:, half * 4 : half * 4 + 4, s * BPC : (s + 1) * BPC],
                        in0=v3(pv)[:, half * 4 : half * 4 + 4, pslot * BPC : (pslot + 1) * BPC],
                        scalar=1.0 - ALPHA,
                        in1=v3(r, nch=4),
                        op0=mybir.AluOpType.mult,
                        op1=mybir.AluOpType.add,
                    )
                    # moving-operand copy (cast when DT_REC != f32)
                    nc.vector.tensor_copy(
                        out=v3(hm_out[half], nch=4),
                        in_=v3(ob_cur)[:, half * 4 : half * 4 + 4, s * BPC : (s + 1) * BPC],
                    )

            def do_half(xi_t, ob_cur, ob_prev, hm_first):
                hm_prev = hm_first
                for s in range(GROUP):
                    prev = (ob_prev, GROUP - 1) if s == 0 else (ob_cur, s - 1)
                    last = s == GROUP - 1
                    if last and ob_cur is ob_B:
                        hm_out = hm_wrap
                    else:
                        hm_out = (
                            hm_pool.tile([128, 4 * BPC], DT_REC, name="hml", tag="hml"),
                            hm_pool.tile([128, 4 * BPC], DT_REC, name="hmh", tag="hmh"),
                        )
                    do_step(xi_t, ob_cur, prev, s, hm_prev, hm_out)
                    hm_prev = hm_out
                return hm_prev

            with tc.For_i(
                0, TB, 2 * GF, hint_engines=(mybir.EngineType.PE,)
            ) as i:
                # prefetch xi for the B half of this iteration
                nc.sync.dma_start(
                    out=sb3(xi_B[:], KC), in_=dr3(xi_tmp[:, bass.ds(i + GF, GF)])
                )
                hm_mid = do_half(xi_A, ob_A, ob_B, hm_wrap)
                nc.sync.dma_start(
                    out=dr3(outT[:, bass.ds(i, GF)]), in_=sb3(ob_A[:], KC)
                )
                # prefetch xi_A for the next iteration
                nc.sync.dma_start(
                    out=sb3(xi_A[:], KC), in_=dr3(xi_tmp[:, bass.ds(i + 2 * GF, GF)])
                )
                do_half(xi_B, ob_B, ob_A, hm_mid)
                nc.sync.dma_start(
                    out=dr3(outT[:, bass.ds(i + GF, GF)]), in_=sb3(ob_B[:], KC)
                )

    return nc


_NC_CACHE = {}


def _get_nc():
    key = (str(DT_REC), str(DT_GEMM))
    if key not in _NC_CACHE:
        _NC_CACHE[key] = build_bass()
    return _NC_CACHE[key]


# -------------------------------------------------------------- host entry
def kernel(x, h0, Wi, bi, Wh, bh):
    x = np.asarray(x, np.float32)
    h0 = np.asarray(h0, np.float32)
    Wi = np.asarray(Wi, np.float32)
    bi = np.asarray(bi, np.float32)
    Wh = np.asarray(Wh, np.float32)
    bh = np.asarray(bh, np.float32)

    np_rec = mybir.dt.np(DT_REC)
    np_gemm = mybir.dt.np(DT_GEMM)

    wiT = np.ascontiguousarray((ALPHA * Wi).T).astype(np_gemm)      # (256, 1024)
    whT = np.ascontiguousarray((ALPHA * Wh).T).astype(np_rec)       # (1024, 1024)
    biasv = (ALPHA * (bi + bh)).reshape(HID, 1).astype(np.float32)  # (1024, 1)

    in_maps = []
    for c in range(N_CORES):
        xs = x[:, c * BPC : (c + 1) * BPC, :].reshape(TB, INSZ)
        xT = np.ascontiguousarray(xs.T)                             # (256, 8192)
        h0T = np.ascontiguousarray(h0[c * BPC : (c + 1) * BPC, :].T)
        in_maps.append(
            {"xT": xT, "wiT": wiT, "whT": whT, "biasv": biasv, "h0T": h0T}
        )

    nc = _get_nc()
    res = run_bass_kernel_spmd(nc, in_maps, core_ids=list(range(N_CORES)))

    output = np.empty((SEQ, BATCH, HID), np.float32)
    for c in range(N_CORES):
        oT = res.results[c]["outT"]                                 # (1024, 8192)
        output[:, c * BPC : (c + 1) * BPC, :] = (
            oT.reshape(HID, SEQ, BPC).transpose(1, 2, 0)
        )
    h_final = output[-1].copy()
    return output, h_final


if __name__ == "__main__":
    nc = build_bass()
    print("build OK")
